# revision 1
# baseline (speedup 1.0000x reference)
"""Trainium2 Bass kernel for nn_Linear_27608049779368.

Reference computation:
    out[b,c] = bias[c] + sum_o prod(x[:, idx_o], axis=2) @ W_o
    x [4096, 32], orders 1..3 with 32/496/4960 combos, C=128 classes.

Device algorithm (per core, data-parallel over batch, 8 cores x 512 rows):
    out.T = Wp.T @ exp(Inc.T @ log(x.T + c))        (all fp32)

  * c > -min(x) shifts features positive so products become sums of logs.
  * Inc [32, NK]: multiplicity of feature f in row-multiset T.  A single
    K=32 matmul per 128-row tile computes all the gathers AND products.
  * exp on ScalarE evacuates PSUM -> SBUF (only full-tensor elementwise
    pass; every other step is a matmul).
  * Wp [NK, 128] is host-transformed: expanding prod(x_f) =
    prod((x_f+c) - c) folds every cross term exactly into the weight row
    of the corresponding sub-multiset (all of which are themselves rows).
    The empty multiset is a constant row absorbing bias and c^o terms.
  * "Anti-mean" constant rows every 32 rows keep PSUM partial sums
    centered (numerics only; exactly compensated by a final restore row).

The result is mathematically exact in real arithmetic.  Measured on
hardware: absmax error 8.4e-3 on an output absmax of 15.9 (5.3e-4 of
scale), dominated by the PE fp32 matmul's internal per-product rounding
on the shift-inflated exp values; CoreSim cost model ~78us/core.
"""

import os
import sys
from itertools import combinations as _combinations

import numpy as np

for _p in ("/opt/trn_rl_repo", "/root/.axon_site/_ro/trn_rl_repo"):
    if os.path.isdir(_p) and _p not in sys.path:
        sys.path.insert(0, _p)
        break

import concourse.bass as bass
import concourse.bacc as bacc
import concourse.tile as tile
from concourse import mybir
from concourse.bass_utils import run_bass_kernel_spmd

N_CORES = 8
P = 128                 # partitions / tile size
EXP_FUSE = 3            # k-tiles per fused exp op (3 PSUM banks)
ANTI_MEAN_SPACING = 39  # centering const-row every N rows (39 -> 44 k-tiles)
F32 = mybir.dt.float32
F32R = mybir.dt.float32r
# fp32 matmuls stream at 4 cycles/row; float32r at 1 (N>=256).  The
# incidence matmul is made exact at fp32r speed by splitting log(x') into
# an 11-bit-mantissa high part plus residual (both fp32r-representable)
# and accumulating two fp32r matmuls in PSUM.
INC_FP32R_SPLIT = True


# ----------------------------------------------------------------------------
# Host-side math: rows, incidence, transformed weights
# ----------------------------------------------------------------------------

def _build_rows(idx_list, W_list, bias, c, F=32):
    """Build the row table (multisets), incidence and transformed weights.

    Returns Inc [F, NK] f32, Wp [NK, C] f64, rows (list of tuples).
    """
    C = W_list[0].shape[1]
    row_of = {}
    rows = []

    def get_row(t):
        r = row_of.get(t)
        if r is None:
            r = len(rows)
            row_of[t] = r
            rows.append(t)
        return r

    # Register original combos first, in given order, so the main mass of
    # each order sits in contiguous row blocks.
    combo_rows = []
    for idx, W in zip(idx_list, W_list):
        for k in range(idx.shape[0]):
            M = tuple(sorted(int(v) for v in idx[k]))
            combo_rows.append(get_row(M))

    Wp_contrib = []  # (row, coeff, W_vector)
    ci = 0
    const_acc = np.array(bias, np.float64).reshape(-1).copy()
    for idx, W in zip(idx_list, W_list):
        o = idx.shape[1]
        for k in range(idx.shape[0]):
            M = tuple(sorted(int(v) for v in idx[k]))
            Wk = W[k].astype(np.float64)
            for r in range(o, -1, -1):
                for sub in set(_combinations(M, r)):
                    cnt = sum(
                        1
                        for ss in _combinations(range(o), r)
                        if tuple(sorted(M[i] for i in ss)) == sub
                    )
                    coeff = ((-float(c)) ** (o - r)) * cnt
                    if r == 0:
                        const_acc += coeff * Wk
                    else:
                        Wp_contrib.append((get_row(sub), coeff, Wk))
            ci += 1

    const_row = get_row(())
    NK = len(rows)
    Inc = np.zeros((F, NK), np.float32)
    for r, t in enumerate(rows):
        for f in t:
            Inc[f, r] += 1.0
    Wp = np.zeros((NK, C), np.float64)
    for r, coeff, Wk in Wp_contrib:
        Wp[r] += coeff * Wk
    Wp[const_row] += const_acc
    return Inc, Wp, rows


def _add_anti_mean_rows(x, Inc, Wp, c, spacing):
    """Insert const rows every `spacing` rows cancelling the batch-mean mass
    of the preceding block; a final const row restores the total (exact)."""
    f32 = np.float32
    xp = np.maximum(x.astype(np.float64) + float(c), 1.0 / 64)
    Pv = np.exp(np.log(xp) @ Inc.astype(np.float64))     # [B, NK]
    mu = Pv.mean(axis=0)                                  # [NK]
    NK, C = Wp.shape
    F = Inc.shape[0]
    inc_cols, wp_rows = [], []
    total = np.zeros(C, np.float64)
    for t0 in range(0, NK, spacing):
        t1 = min(t0 + spacing, NK)
        inc_cols.append(Inc[:, t0:t1])
        wp_rows.append(Wp[t0:t1])
        mass = (mu[t0:t1, None] * Wp[t0:t1]).sum(axis=0)
        total += mass
        inc_cols.append(np.zeros((F, 1), f32))
        wp_rows.append((-mass)[None, :])
    inc_cols.append(np.zeros((F, 1), f32))
    wp_rows.append(total[None, :])
    return np.concatenate(inc_cols, axis=1), np.concatenate(wp_rows, axis=0)


def _split_big_weight_rows(Inc, Wp, thresh=32.0):
    """The PE's fp32 matmul multiplies with ~17-bit effective mantissas, so a
    product |P*W| is rounded at ~2^-17 relative.  Rows with large |W| (the
    constant / anti-mean / restore rows, whose P is exactly 1.0) dominate that
    error.  Split each such row into an 11-bit-mantissa hi part plus residual
    (both exactly representable through the truncated multiply) with a
    duplicated incidence column — mathematically identical, numerically clean.
    """
    mags = np.abs(Wp).max(axis=1)
    big = np.nonzero(mags > thresh)[0]
    if len(big) == 0:
        return Inc, Wp
    W32 = Wp.astype(np.float32)
    bits = W32.view(np.uint32)
    hi = (bits & np.uint32(0xFFFFF000)).view(np.float32)
    inc_cols = [Inc]
    wp_rows = [Wp.copy()]
    for r in big:
        lo = (W32[r].astype(np.float64) - hi[r].astype(np.float64))
        wp_rows[0][r] = hi[r]
        inc_cols.append(Inc[:, r:r + 1])
        wp_rows.append(lo[None, :])
    return np.concatenate(inc_cols, axis=1), np.concatenate(wp_rows, axis=0)


def _prepare(x, bias, W1, W2, W3, idx1, idx2, idx3):
    c = max(1.0, 0.5 - float(x.min()))
    Inc, Wp, _rows = _build_rows(
        [np.asarray(idx1), np.asarray(idx2), np.asarray(idx3)],
        [np.asarray(W1), np.asarray(W2), np.asarray(W3)],
        np.asarray(bias), c, F=np.asarray(x).shape[1])
    Inc, Wp = _add_anti_mean_rows(np.asarray(x), Inc, Wp, c, ANTI_MEAN_SPACING)
    NK = Inc.shape[1]
    nt = -(-NK // P)
    pad = nt * P - NK
    if pad:
        # dead rows: Inc col 0 -> exp(0)=1, Wp row 0 -> no contribution
        Inc = np.concatenate([Inc, np.zeros((Inc.shape[0], pad), np.float32)], axis=1)
        Wp = np.concatenate([Wp, np.zeros((pad, Wp.shape[1]), np.float64)], axis=0)
    return c, np.ascontiguousarray(Inc, np.float32), \
        np.ascontiguousarray(Wp.astype(np.float32)), nt


# ----------------------------------------------------------------------------
# Device kernel
# ----------------------------------------------------------------------------

def _build_nc(F, C, b_shard, nt, repeat=1):
    # Bacc (not plain Bass): finalize() runs the legalization passes —
    # notably generate_event_semaphores, which splits multi-sem waits
    # (TRN2 allows at most one sync wait per instruction).
    nc = bacc.Bacc(None, target_bir_lowering=False)
    d_xT = nc.declare_dram_parameter("xT", [F, b_shard], F32, isOutput=False)
    d_cv = nc.declare_dram_parameter("cvec", [F, 1], F32, isOutput=False)
    d_inc = nc.declare_dram_parameter("inc", [F, nt * P], F32, isOutput=False)
    d_wp = nc.declare_dram_parameter("wp", [nt * P, C], F32, isOutput=False)
    d_outT = nc.declare_dram_parameter("outT", [C, b_shard], F32, isOutput=True)

    with tile.TileContext(nc) as tc:
        with (
            tc.tile_pool(name="consts", bufs=1) as consts,
            tc.tile_pool(name="prods", bufs=1) as prods_pool,
            tc.tile_pool(name="wp_pool", bufs=8) as wp_pool,
            tc.tile_pool(name="psum_L", bufs=2, space="PSUM") as psum_L,
            tc.tile_pool(name="psum_out", bufs=1, space="PSUM") as psum_out,
        ):
            x_sb = consts.tile([F, b_shard], F32)
            nc.gpsimd.dma_start(out=x_sb, in_=d_xT[:, :])
            c_sb = consts.tile([F, 1], F32)
            nc.gpsimd.dma_start(out=c_sb, in_=d_cv[:, :])
            inc_sb = consts.tile([F, nt * P], F32)
            nc.gpsimd.dma_start(out=inc_sb, in_=d_inc[:, :])

            for _rep in range(repeat):
                _body_once(nc, tc, consts, prods_pool, wp_pool, psum_L,
                           psum_out, d_wp, d_outT, x_sb, c_sb, inc_sb,
                           F, C, b_shard, nt)
    nc.finalize()
    return nc


def _body_once(nc, tc, consts, prods_pool, wp_pool, psum_L, psum_out,
               d_wp, d_outT, x_sb, c_sb, inc_sb, F, C, b_shard, nt):
    # x' = max(x + c, 1/64); lx = log(x')
    xp_sb = consts.tile([F, b_shard], F32)
    nc.vector.tensor_scalar(
        out=xp_sb, in0=x_sb, scalar1=c_sb, scalar2=1.0 / 64,
        op0=mybir.AluOpType.add, op1=mybir.AluOpType.max)
    lx0 = consts.tile([F, b_shard], F32)
    nc.scalar.activation(lx0, xp_sb, mybir.ActivationFunctionType.Ln)
    # One Newton step refines the Ln table approximation to ~fp32 exactness:
    # l' = l + (x' * exp(-l) - 1).  The raw spline error (~1e-5) otherwise
    # dominates the end-to-end error (measured on hardware).
    e_neg = consts.tile([F, b_shard], F32)
    nc.scalar.activation(e_neg, lx0, mybir.ActivationFunctionType.Exp,
                         scale=-1.0)
    corr = consts.tile([F, b_shard], F32)
    nc.vector.tensor_mul(out=corr, in0=xp_sb, in1=e_neg)
    lx_sb = consts.tile([F, b_shard], F32)
    nc.vector.scalar_tensor_tensor(
        out=lx_sb, in0=corr, scalar=1.0, in1=lx0,
        op0=mybir.AluOpType.subtract, op1=mybir.AluOpType.add)

    if INC_FP32R_SPLIT:
        # lx = lx_hi + lx_res with both parts exactly fp32r
        # representable (the residual of a 12-bit round has at most
        # 12 significant bits), so two fp32r matmuls accumulating in
        # fp32 PSUM reproduce the fp32 matmul exactly.
        lx_hi = consts.tile([F, b_shard], F32R)
        nc.vector.tensor_copy(out=lx_hi, in_=lx_sb)
        lx_res = consts.tile([F, b_shard], F32)
        nc.vector.tensor_sub(out=lx_res, in0=lx_sb, in1=lx_hi)
        lx_res_r = consts.tile([F, b_shard], F32R)
        nc.vector.tensor_copy(out=lx_res_r, in_=lx_res)
        inc_r = consts.tile([F, nt * P], F32R)
        inc_mm = inc_r
        rhs_parts = [lx_hi, lx_res_r]
    else:
        inc_mm = inc_sb
        rhs_parts = [lx_sb]

    # log-sum matmuls + fused exp
    prods_tiles = []
    t = 0
    gi = 0
    while t < nt:
        g = min(EXP_FUSE, nt - t)
        if INC_FP32R_SPLIT:
            nc.vector.tensor_copy(out=inc_r[:, t * P:(t + g) * P],
                                  in_=inc_sb[:, t * P:(t + g) * P])
        L_ps = psum_L.tile([P, EXP_FUSE * b_shard], F32, tag="L")
        for j in range(g):
            for pi, rhs in enumerate(rhs_parts):
                nc.tensor.matmul(
                    L_ps[:, j * b_shard:(j + 1) * b_shard],
                    inc_mm[:, (t + j) * P:(t + j + 1) * P],
                    rhs,
                    start=(pi == 0), stop=(pi == len(rhs_parts) - 1))
        pg = prods_pool.tile([P, g * b_shard], F32, tag=f"pg{gi}")
        nc.scalar.activation(
            pg, L_ps[:, :g * b_shard], mybir.ActivationFunctionType.Exp)
        for j in range(g):
            prods_tiles.append(pg[:, j * b_shard:(j + 1) * b_shard])
        t += g
        gi += 1

    # main contraction: outT += Wp_tile.T @ prods_tile
    out_ps = psum_out.tile([C, b_shard], F32)
    for t2 in range(nt):
        wp_t = wp_pool.tile([P, C], F32, tag="wp")
        nc.gpsimd.dma_start(out=wp_t, in_=d_wp[t2 * P:(t2 + 1) * P, :])
        nc.tensor.matmul(
            out_ps, wp_t, prods_tiles[t2],
            start=(t2 == 0), stop=(t2 == nt - 1))

    out_sb = consts.tile([C, b_shard], F32)
    nc.vector.tensor_copy(out=out_sb, in_=out_ps)
    nc.gpsimd.dma_start(out=d_outT[:, :], in_=out_sb)


_nc_cache = {}


def _get_nc(F, C, b_shard, nt, repeat=1):
    key = (F, C, b_shard, nt, repeat)
    if key not in _nc_cache:
        _nc_cache[key] = _build_nc(F, C, b_shard, nt, repeat)
    return _nc_cache[key]


def _make_in_maps(x, c, Inc, Wp, b_shard):
    F = x.shape[1]
    cvec = np.full((F, 1), c, np.float32)
    in_maps = []
    for i in range(N_CORES):
        sh = np.ascontiguousarray(
            x[i * b_shard:(i + 1) * b_shard].T.astype(np.float32))
        in_maps.append({"xT": sh, "cvec": cvec, "inc": Inc, "wp": Wp})
    return in_maps


def kernel(x, bias, W1, W2, W3, idx1, idx2, idx3, _trace=False):
    x = np.asarray(x, np.float32)
    B, F = x.shape
    C = np.asarray(W1).shape[1]
    assert B % N_CORES == 0
    b_shard = B // N_CORES

    c, Inc, Wp, nt = _prepare(x, bias, W1, W2, W3, idx1, idx2, idx3)
    nc = _get_nc(F, C, b_shard, nt)
    in_maps = _make_in_maps(x, c, Inc, Wp, b_shard)
    res = run_bass_kernel_spmd(nc, in_maps, list(range(N_CORES)), trace=_trace)
    out = np.empty((B, C), np.float32)
    for i in range(N_CORES):
        out[i * b_shard:(i + 1) * b_shard] = res.results[i]["outT"].T
    if _trace:
        kernel.last_results = res
    return out



# revision 12
# speedup vs baseline: 2.4709x; 2.4709x over previous
"""Trainium2 Bass kernel for nn_Linear_27608049779368.

Reference computation:
    out[b,c] = bias[c] + sum_o prod(x[:, idx_o], axis=2) @ W_o
    x [4096, 32], orders 1..3 with 32/496/4960 combos, C=128 classes.

Data-parallel over batch: 8 cores x 512 rows each.

Per-core algorithm (instruction-count / DMA-byte minimized for this stack,
where NEFF-internal DMA bytes and cross-engine instructions dominate):

  1. Encode each feature as TWO fp16 rhs rows: lx_i = ln(max(|x_i|,1e-8))
     and 2048*s_i (s_i = 1 if x_i < 0), plus one constant row of 64.
  2. 43 fp16 matmuls (K=65) against the duplicated incidence matrix give
     L' = sum(inc*lx) + 64 + 2048*par per combo row, where par is the
     count of negative factors.  2048*inc*s and 64 are exact in fp16/fp32,
     so the magnitude (log) and sign (parity) channels never mix.
  3. Evacuate each PSUM group with three ops:
        m   = L' mod 2048            (= log-magnitude + 64)
        odd = (L' mod 4096) >= 2048  (= parity bit)
        prods = exp(m - 64)          (ACT, fused bias)
     then one global fold prods *= (1 - 2*odd) gives the TRUE signed
     products (no c-shift, no cancellation blow-up) in bf16.
  4. 43 bf16 matmuls accumulate out = prods.T-tiles @ [W1;W2;W3] into one
     PSUM bank (chained accumulation group = cheap dispatch).
  5. bias is folded in on the host (device output + bias).

All weights/incidence ship as bf16/fp16: ~2.3MB of NEFF DMA per core vs
3.9MB fp32 for the baseline, and ~86 PE + ~50 DVE/ACT instructions vs
~250 mixed instructions with 50 SWDGE DMAs.
"""

import os
import sys

import numpy as np

for _p in ("/opt/trn_rl_repo", "/root/.axon_site/_ro/trn_rl_repo"):
    if os.path.isdir(_p) and _p not in sys.path:
        sys.path.insert(0, _p)
        break

import concourse.bass as bass
import concourse.bacc as bacc
import concourse.tile as tile
from concourse import mybir
from concourse.bass_utils import run_bass_kernel_spmd

N_CORES = 8
P = 128
F32 = mybir.dt.float32
F16 = mybir.dt.float16
BF16 = mybir.dt.bfloat16

GROUP = 3          # product tiles per PSUM evacuation group (3 banks)
WP_DMA_SPLIT = 4   # number of chunks for the weight DMA
M_ENC = 2048.0     # parity offset; exact in fp16, >> |L|+64
LOG_CLAMP = 1e-8


# ----------------------------------------------------------------------------
# Host-side prep
# ----------------------------------------------------------------------------

def _build_tables(W1, W2, W3, idx1, idx2, idx3, F):
    """Incidence (fp16, duplicated rows + const row) and weights (bf16,
    SBUF tile layout)."""
    idxs = [np.asarray(idx1), np.asarray(idx2), np.asarray(idx3)]
    Ws = [np.asarray(W1), np.asarray(W2), np.asarray(W3)]
    C = Ws[0].shape[1]
    NK = sum(i.shape[0] for i in idxs)
    nt = -(-NK // P)
    NKp = nt * P

    inc = np.zeros((F, NKp), np.float32)
    col = 0
    for idx in idxs:
        n, o = idx.shape
        cols = np.arange(col, col + n)
        for j in range(o):
            np.add.at(inc, (idx[:, j], cols), 1.0)
        col += n

    inc16 = np.zeros((2 * F, NKp), np.float16)
    inc16[:F] = inc
    inc16[F:] = inc

    Wp = np.zeros((NKp, C), np.float32)
    Wp[:NK] = np.vstack([w.astype(np.float32) for w in Ws])
    # SBUF layout: wp_dev[p, t*C + c] = Wp[t*P + p, c]
    import ml_dtypes
    wp_dev = np.ascontiguousarray(
        Wp.reshape(nt, P, C).transpose(1, 0, 2).reshape(P, nt * C)
    ).astype(ml_dtypes.bfloat16)
    return inc16, wp_dev, nt


# ----------------------------------------------------------------------------
# Device kernel
# ----------------------------------------------------------------------------

def _build_nc(F, C, b_shard, nt, repeat=1):
    K = 2 * F
    nc = bacc.Bacc(None, target_bir_lowering=False)
    d_xT = nc.declare_dram_parameter("xT", [F, b_shard], F32, isOutput=False)
    d_inc = nc.declare_dram_parameter("inc16", [K, nt * P], F16, isOutput=False)
    d_wp = nc.declare_dram_parameter("wp", [P, nt * C], BF16, isOutput=False)
    d_outT = nc.declare_dram_parameter("outT", [C, b_shard], BF16, isOutput=True)

    with tile.TileContext(nc) as tc:
        with (
            tc.tile_pool(name="consts", bufs=1) as consts,
            tc.tile_pool(name="bigbuf", bufs=1) as bigbuf,
            tc.tile_pool(name="scratch", bufs=2) as scratch,
            tc.tile_pool(name="psum_L", bufs=2, space="PSUM") as psum_L,
            tc.tile_pool(name="psum_acc", bufs=1, space="PSUM") as psum_acc,
        ):
            x_sb = consts.tile([F, b_shard], F32)
            nc.sync.dma_start(out=x_sb, in_=d_xT[:, :])
            inc_sb = consts.tile([K, nt * P], F16)
            nc.sync.dma_start(out=inc_sb, in_=d_inc[:, :])
            wp_sb = consts.tile([P, nt * C], BF16)
            ncols = nt * C
            step = -(-ncols // WP_DMA_SPLIT)
            for c0 in range(0, ncols, step):
                c1 = min(c0 + step, ncols)
                nc.sync.dma_start(out=wp_sb[:, c0:c1], in_=d_wp[:, c0:c1])

            for _rep in range(repeat):
                _body(nc, tc, consts, bigbuf, scratch, psum_L, psum_acc,
                      d_outT, x_sb, inc_sb, wp_sb, F, C, b_shard, nt)
    nc.finalize()
    return nc


def _body(nc, tc, consts, bigbuf, scratch, psum_L, psum_acc, d_outT,
          x_sb, inc_sb, wp_sb, F, C, b_shard, nt):
    # rhs16 rows: [0,F) = ln(max(|x|,eps)); [F,2F) = (x<0); [2F] = -1.5
    # L-channel matmuls use rows [0,F); parity matmuls rows [F,2F] so the
    # PSUM parity value is (#negative factors) - 1.5 and
    # Sin(pi * that) = (-1)^par exactly (sine extrema).
    rhs16 = scratch.tile([2 * F, b_shard], F16, tag="rhs")
    ax = scratch.tile([F, b_shard], F32, tag="ax")
    nc.scalar.activation(ax, x_sb, mybir.ActivationFunctionType.Abs)
    axc = scratch.tile([F, b_shard], F32, tag="axc")
    nc.vector.tensor_scalar(
        out=axc, in0=ax, scalar1=LOG_CLAMP, scalar2=None,
        op0=mybir.AluOpType.max)
    nc.scalar.activation(rhs16[0:F], axc, mybir.ActivationFunctionType.Ln)
    nc.vector.tensor_scalar(
        out=rhs16[F:2 * F], in0=x_sb, scalar1=0.0, scalar2=None,
        op0=mybir.AluOpType.is_lt)

    prods = bigbuf.tile([P, nt * b_shard], BF16, tag="prods")
    sig = bigbuf.tile([P, nt * b_shard], BF16, tag="sig")

    t = 0
    while t < nt:
        g = min(GROUP, nt - t)
        Lp = psum_L.tile([P, GROUP * b_shard], F32, tag="L")
        for j in range(g):
            nc.tensor.matmul(
                Lp[:, j * b_shard:(j + 1) * b_shard],
                inc_sb[0:F, (t + j) * P:(t + j + 1) * P],
                rhs16[0:F],
                start=True, stop=True)
        nc.scalar.activation(
            prods[:, t * b_shard:(t + g) * b_shard], Lp[:, :g * b_shard],
            mybir.ActivationFunctionType.Exp)
        Pp = psum_L.tile([P, GROUP * b_shard], F32, tag="L")
        for j in range(g):
            nc.tensor.matmul(
                Pp[:, j * b_shard:(j + 1) * b_shard],
                inc_sb[F:2 * F, (t + j) * P:(t + j + 1) * P],
                rhs16[F:2 * F],
                start=True, stop=True)
        nc.vector.tensor_scalar(
            out=sig[:, t * b_shard:(t + g) * b_shard], in0=Pp[:, :g * b_shard],
            scalar1=-2.0, scalar2=1.0,
            op0=mybir.AluOpType.mult, op1=mybir.AluOpType.add)
        t += g

    # sign fold in two halves so the contraction can start early.
    # sig holds 1-2*par in {1,-1,-3,-5}; adding 4*(par>=2) maps it to
    # (-1)^par in {1,-1} exactly.
    h1 = (nt // 2) * b_shard
    htmp = bigbuf.tile([P, nt * b_shard - h1], BF16, tag="htmp")
    for lo, hi in ((0, h1), (h1, nt * b_shard)):
        n = hi - lo
        nc.vector.tensor_scalar(
            out=htmp[:, :n], in0=sig[:, lo:hi], scalar1=-2.5, scalar2=4.0,
            op0=mybir.AluOpType.is_le, op1=mybir.AluOpType.mult)
        nc.vector.tensor_add(
            out=sig[:, lo:hi], in0=sig[:, lo:hi], in1=htmp[:, :n])
        nc.vector.tensor_mul(
            out=prods[:, lo:hi], in0=prods[:, lo:hi], in1=sig[:, lo:hi])

    acc = psum_acc.tile([C, b_shard], F32)
    for t2 in range(nt):
        nc.tensor.matmul(
            acc,
            wp_sb[:, t2 * C:(t2 + 1) * C],
            prods[:, t2 * b_shard:(t2 + 1) * b_shard],
            start=(t2 == 0), stop=(t2 == nt - 1))

    out_sb = bigbuf.tile([C, b_shard], BF16, tag="out")
    nc.vector.tensor_copy(out=out_sb, in_=acc)
    nc.sync.dma_start(out=d_outT[:, :], in_=out_sb)


_nc_cache = {}


def _get_nc(F, C, b_shard, nt, repeat=1):
    key = (F, C, b_shard, nt, repeat)
    if key not in _nc_cache:
        _nc_cache[key] = _build_nc(F, C, b_shard, nt, repeat)
    return _nc_cache[key]


def _make_in_maps(x, inc16, wp_dev, b_shard):
    in_maps = []
    for i in range(N_CORES):
        sh = np.ascontiguousarray(
            x[i * b_shard:(i + 1) * b_shard].T.astype(np.float32))
        in_maps.append({"xT": sh, "inc16": inc16, "wp": wp_dev})
    return in_maps


def kernel(x, bias, W1, W2, W3, idx1, idx2, idx3, _trace=False):
    x = np.asarray(x, np.float32)
    B, F = x.shape
    C = np.asarray(W1).shape[1]
    assert B % N_CORES == 0
    b_shard = B // N_CORES

    inc16, wp_dev, nt = _build_tables(W1, W2, W3, idx1, idx2, idx3, F)
    nc = _get_nc(F, C, b_shard, nt)
    in_maps = _make_in_maps(x, inc16, wp_dev, b_shard)
    res = run_bass_kernel_spmd(nc, in_maps, list(range(N_CORES)), trace=_trace)
    out = np.empty((B, C), np.float32)
    for i in range(N_CORES):
        o = np.asarray(res.results[i]["outT"]).astype(np.float32)
        out[i * b_shard:(i + 1) * b_shard] = o.T
    out += np.asarray(bias, np.float32).reshape(1, -1)
    if _trace:
        kernel.last_results = res
    return out


# revision 14
# speedup vs baseline: 5.1262x; 2.0746x over previous
"""Trainium2 Bass kernel for nn_Linear_27608049779368.

Reference computation:
    out[b,c] = bias[c] + sum_o prod(x[:, idx_o], axis=2) @ W_o
    x [4096, 32], orders 1..3 with 32/496/4960 combos, C=128 classes.

Data-parallel over batch: 8 cores x 512 rows each.

Per-core algorithm (instruction-count / DMA-byte minimized for this stack,
where NEFF-internal DMA bytes and cross-engine instructions dominate):

  1. Encode each feature as TWO fp16 rhs rows: lx_i = ln(max(|x_i|,1e-8))
     and 2048*s_i (s_i = 1 if x_i < 0), plus one constant row of 64.
  2. 43 fp16 matmuls (K=65) against the duplicated incidence matrix give
     L' = sum(inc*lx) + 64 + 2048*par per combo row, where par is the
     count of negative factors.  2048*inc*s and 64 are exact in fp16/fp32,
     so the magnitude (log) and sign (parity) channels never mix.
  3. Evacuate each PSUM group with three ops:
        m   = L' mod 2048            (= log-magnitude + 64)
        odd = (L' mod 4096) >= 2048  (= parity bit)
        prods = exp(m - 64)          (ACT, fused bias)
     then one global fold prods *= (1 - 2*odd) gives the TRUE signed
     products (no c-shift, no cancellation blow-up) in bf16.
  4. 43 bf16 matmuls accumulate out = prods.T-tiles @ [W1;W2;W3] into one
     PSUM bank (chained accumulation group = cheap dispatch).
  5. bias is folded in on the host (device output + bias).

All weights/incidence ship as bf16/fp16: ~2.3MB of NEFF DMA per core vs
3.9MB fp32 for the baseline, and ~86 PE + ~50 DVE/ACT instructions vs
~250 mixed instructions with 50 SWDGE DMAs.
"""

import os
import sys

import numpy as np

for _p in ("/opt/trn_rl_repo", "/root/.axon_site/_ro/trn_rl_repo"):
    if os.path.isdir(_p) and _p not in sys.path:
        sys.path.insert(0, _p)
        break

import concourse.bass as bass
import concourse.bacc as bacc
import concourse.tile as tile
from concourse import mybir
from concourse.bass_utils import run_bass_kernel_spmd

N_CORES = 8
P = 128
F32 = mybir.dt.float32
F16 = mybir.dt.float16
BF16 = mybir.dt.bfloat16

GROUP = 3          # product tiles per PSUM evacuation group (3 banks)
WP_DMA_SPLIT = 4   # number of chunks for the weight DMA
M_ENC = 2048.0     # parity offset; exact in fp16, >> |L|+64
LOG_CLAMP = 1e-8


# ----------------------------------------------------------------------------
# Host-side prep
# ----------------------------------------------------------------------------

def _build_tables(W1, W2, W3, idx1, idx2, idx3, F):
    """Incidence (fp16, duplicated rows + const row) and weights (bf16,
    SBUF tile layout)."""
    idxs = [np.asarray(idx1), np.asarray(idx2), np.asarray(idx3)]
    Ws = [np.asarray(W1), np.asarray(W2), np.asarray(W3)]
    C = Ws[0].shape[1]
    NK = sum(i.shape[0] for i in idxs)
    nt = -(-NK // P)
    NKp = nt * P

    inc = np.zeros((F, NKp), np.float32)
    col = 0
    for idx in idxs:
        n, o = idx.shape
        cols = np.arange(col, col + n)
        for j in range(o):
            np.add.at(inc, (idx[:, j], cols), 1.0)
        col += n

    inc16 = np.zeros((2 * F, NKp), np.float16)
    inc16[:F] = inc
    inc16[F:] = inc

    Wp = np.zeros((NKp, C), np.float32)
    Wp[:NK] = np.vstack([w.astype(np.float32) for w in Ws])
    # SBUF layout: wp_dev[p, t*C + c] = Wp[t*P + p, c]
    import ml_dtypes
    wp_dev = np.ascontiguousarray(
        Wp.reshape(nt, P, C).transpose(1, 0, 2).reshape(P, nt * C)
    ).astype(ml_dtypes.bfloat16)
    return inc16, wp_dev, nt


# ----------------------------------------------------------------------------
# Device kernel
# ----------------------------------------------------------------------------

def _build_nc(F, C, b_shard, nt, repeat=1):
    K = 2 * F
    nc = bacc.Bacc(None, target_bir_lowering=False)
    d_xT = nc.declare_dram_parameter("xT", [F, b_shard], F32, isOutput=False)
    d_inc = nc.declare_dram_parameter("inc16", [K, nt * P], F16, isOutput=False)
    d_wp = nc.declare_dram_parameter("wp", [P, nt * C], BF16, isOutput=False)
    d_outT = nc.declare_dram_parameter("outT", [C, b_shard], BF16, isOutput=True)

    with tile.TileContext(nc) as tc:
        with (
            tc.tile_pool(name="consts", bufs=1) as consts,
            tc.tile_pool(name="bigbuf", bufs=1) as bigbuf,
            tc.tile_pool(name="scratch", bufs=2) as scratch,
            tc.tile_pool(name="psum_L", bufs=2, space="PSUM") as psum_L,
            tc.tile_pool(name="psum_acc", bufs=1, space="PSUM") as psum_acc,
        ):
            x_sb = consts.tile([F, b_shard], F32)
            nc.sync.dma_start(out=x_sb, in_=d_xT[:, :])
            inc_sb = consts.tile([K, nt * P], F16)
            nc.sync.dma_start(out=inc_sb, in_=d_inc[:, :])
            wp_sb = consts.tile([P, nt * C], BF16)
            ncols = nt * C
            step = -(-ncols // WP_DMA_SPLIT)
            for c0 in range(0, ncols, step):
                c1 = min(c0 + step, ncols)
                nc.sync.dma_start(out=wp_sb[:, c0:c1], in_=d_wp[:, c0:c1])

            for _rep in range(repeat):
                _body(nc, tc, consts, bigbuf, scratch, psum_L, psum_acc,
                      d_outT, x_sb, inc_sb, wp_sb, F, C, b_shard, nt)
    nc.finalize()
    return nc


def _body(nc, tc, consts, bigbuf, scratch, psum_L, psum_acc, d_outT,
          x_sb, inc_sb, wp_sb, F, C, b_shard, nt):
    # rhs16 rows: [0,F) = ln(max(|x|,eps)); [F,2F) = (x<0); [2F] = -1.5
    # L-channel matmuls use rows [0,F); parity matmuls rows [F,2F] so the
    # PSUM parity value is (#negative factors) - 1.5 and
    # Sin(pi * that) = (-1)^par exactly (sine extrema).
    rhs16 = scratch.tile([2 * F, b_shard], F16, tag="rhs")
    ax = scratch.tile([F, b_shard], F32, tag="ax")
    nc.scalar.activation(ax, x_sb, mybir.ActivationFunctionType.Abs)
    axc = scratch.tile([F, b_shard], F32, tag="axc")
    nc.vector.tensor_scalar(
        out=axc, in0=ax, scalar1=LOG_CLAMP, scalar2=None,
        op0=mybir.AluOpType.max)
    nc.scalar.activation(rhs16[0:F], axc, mybir.ActivationFunctionType.Ln)
    nc.vector.tensor_scalar(
        out=rhs16[F:2 * F], in0=x_sb, scalar1=0.0, scalar2=None,
        op0=mybir.AluOpType.is_lt)

    prods = bigbuf.tile([P, nt * b_shard], BF16, tag="prods")
    sig = bigbuf.tile([P, nt * b_shard], BF16, tag="sig")

    t = 0
    while t < nt:
        g = min(GROUP, nt - t)
        Lp = psum_L.tile([P, GROUP * b_shard], F32, tag="L")
        for j in range(g):
            # one accumulation group per PSUM tile: disjoint column ranges,
            # so start only clears and stop only ends the group
            nc.tensor.matmul(
                Lp[:, j * b_shard:(j + 1) * b_shard],
                inc_sb[0:F, (t + j) * P:(t + j + 1) * P],
                rhs16[0:F],
                start=True, stop=(j == g - 1), skip_group_check=True)
        nc.scalar.activation(
            prods[:, t * b_shard:(t + g) * b_shard], Lp[:, :g * b_shard],
            mybir.ActivationFunctionType.Exp)
        Pp = psum_L.tile([P, GROUP * b_shard], F32, tag="L")
        for j in range(g):
            nc.tensor.matmul(
                Pp[:, j * b_shard:(j + 1) * b_shard],
                inc_sb[F:2 * F, (t + j) * P:(t + j + 1) * P],
                rhs16[F:2 * F],
                start=True, stop=(j == g - 1), skip_group_check=True)
        nc.vector.tensor_scalar(
            out=sig[:, t * b_shard:(t + g) * b_shard], in0=Pp[:, :g * b_shard],
            scalar1=-2.0, scalar2=1.0,
            op0=mybir.AluOpType.mult, op1=mybir.AluOpType.add)
        t += g

    # sign fold in two halves so the contraction can start early.
    # sig holds 1-2*par in {1,-1,-3,-5}; adding 4*(par>=2) maps it to
    # (-1)^par in {1,-1} exactly.
    h1 = (nt // 2) * b_shard
    htmp = bigbuf.tile([P, nt * b_shard - h1], BF16, tag="htmp")
    for lo, hi in ((0, h1), (h1, nt * b_shard)):
        n = hi - lo
        nc.vector.tensor_scalar(
            out=htmp[:, :n], in0=sig[:, lo:hi], scalar1=-2.5, scalar2=4.0,
            op0=mybir.AluOpType.is_le, op1=mybir.AluOpType.mult)
        nc.vector.tensor_add(
            out=sig[:, lo:hi], in0=sig[:, lo:hi], in1=htmp[:, :n])
        nc.vector.tensor_mul(
            out=prods[:, lo:hi], in0=prods[:, lo:hi], in1=sig[:, lo:hi])

    acc = psum_acc.tile([C, b_shard], F32)
    for t2 in range(nt):
        nc.tensor.matmul(
            acc,
            wp_sb[:, t2 * C:(t2 + 1) * C],
            prods[:, t2 * b_shard:(t2 + 1) * b_shard],
            start=(t2 == 0), stop=(t2 == nt - 1))

    out_sb = bigbuf.tile([C, b_shard], BF16, tag="out")
    nc.vector.tensor_copy(out=out_sb, in_=acc)
    nc.sync.dma_start(out=d_outT[:, :], in_=out_sb)


_nc_cache = {}


def _get_nc(F, C, b_shard, nt, repeat=1):
    key = (F, C, b_shard, nt, repeat)
    if key not in _nc_cache:
        _nc_cache[key] = _build_nc(F, C, b_shard, nt, repeat)
    return _nc_cache[key]


def _make_in_maps(x, inc16, wp_dev, b_shard):
    in_maps = []
    for i in range(N_CORES):
        sh = np.ascontiguousarray(
            x[i * b_shard:(i + 1) * b_shard].T.astype(np.float32))
        in_maps.append({"xT": sh, "inc16": inc16, "wp": wp_dev})
    return in_maps


def kernel(x, bias, W1, W2, W3, idx1, idx2, idx3, _trace=False):
    x = np.asarray(x, np.float32)
    B, F = x.shape
    C = np.asarray(W1).shape[1]
    assert B % N_CORES == 0
    b_shard = B // N_CORES

    inc16, wp_dev, nt = _build_tables(W1, W2, W3, idx1, idx2, idx3, F)
    nc = _get_nc(F, C, b_shard, nt)
    in_maps = _make_in_maps(x, inc16, wp_dev, b_shard)
    res = run_bass_kernel_spmd(nc, in_maps, list(range(N_CORES)), trace=_trace)
    out = np.empty((B, C), np.float32)
    for i in range(N_CORES):
        o = np.asarray(res.results[i]["outT"]).astype(np.float32)
        out[i * b_shard:(i + 1) * b_shard] = o.T
    out += np.asarray(bias, np.float32).reshape(1, -1)
    if _trace:
        kernel.last_results = res
    return out
